# revision 15
# baseline (speedup 1.0000x reference)
"""GAT influence layer on 8 Trainium2 NeuronCores (Bass/Tile), bf16.

Strategy (edge-parallel, dest-block sharded):
  Pass 1 (device): each core computes its 12.5k-node slice of
      Wh = h @ W, s_src = Wh @ a_src, s_dst = Wh @ a_dst  (bf16 matmuls
      against an augmented weight matrix; bf16 in/out streams).
  Host: buckets edges by 32-node destination block; permutes blocks onto
      (core, group, lane) so the 32 blocks sharing a group index have
      similar edge counts; builds per-core bf16 streams: gathered
      messages WhA[col] (65 wide incl. a ones column for the softmax
      denominator), pair-duplicated attention logits q=s_src[row]+
      s_dst[col], and pair-duplicated within-block row indices.
      Data movement only.
  Pass 2 (device): exp(leakyrelu(q)) on ScalarE; a batched one-hot
      selection matrix M = (iota==rr)*exp via two DVE tensor_tensor ops
      (pair-duplicated operands keep the 2x_1P perf mode); the
      softmax-weighted segment-sum as PSUM-accumulated TensorE matmuls,
      4 blocks per 128-row PSUM tile via col-tiling (tile_position);
      deferred division by the per-node denominator (the reference's
      global max-subtract cancels analytically in the softmax).
  Host: un-permutes per-core node-partitioned outputs.
"""

import os
import numpy as np
import ml_dtypes

BF16 = ml_dtypes.bfloat16

N_NODES = 100000
N_EDGES = 1600000
IN_DIM = 128
OUT_DIM = 64
NEG_SLOPE = 0.2
CORES = 8
NPC = N_NODES // CORES          # nodes per core (12500)
BW = 32                         # nodes per block
NLANE = 4                       # blocks (lanes) per group = one PSUM tile
NGRP = 98                       # groups per core; 8*98*4*32 = 100352 >= 1e5
NLANES_G = CORES * NLANE        # 32 blocks share one group index
NGB = CORES * NGRP * NLANE      # 3136 global lane slots (3125 real)
NPP = 12544                     # padded nodes per core, pass 1 (98*128)
SBG = 6                         # groups per superblock (pass-2 stage)
PAD_Q = -30000.0                # pad-slot logit -> exp == 0
W65 = OUT_DIM + 1

LAST_STATS = {}


def _build_pass1():
    from concourse import bacc, mybir
    import concourse.tile as tile

    bf = mybir.dt.bfloat16
    nc = bacc.Bacc("TRN2", target_bir_lowering=False, debug=False)
    d_hT = nc.dram_tensor("hT", [128, NPP], bf, kind="ExternalInput")
    d_W = nc.dram_tensor("Wm", [IN_DIM, OUT_DIM], bf, kind="ExternalInput")
    d_WT = nc.dram_tensor("WT", [OUT_DIM, IN_DIM], bf, kind="ExternalInput")
    d_a2 = nc.dram_tensor("a2", [OUT_DIM, 2], bf, kind="ExternalInput")
    d_whT = nc.dram_tensor("whT", [OUT_DIM + 2, NPP], bf, kind="ExternalOutput")

    NW = 512                    # moving-operand chunk (one PSUM bank fp32)
    with tile.TileContext(nc) as tc:
        with tc.tile_pool(name="c1", bufs=1) as cp, \
             tc.tile_pool(name="ht1", bufs=4) as hp, \
             tc.tile_pool(name="wo1", bufs=4) as wo, \
             tc.tile_pool(name="psw", bufs=1, space="PSUM") as psw, \
             tc.tile_pool(name="ps1", bufs=6, space="PSUM") as psp:
            CHW = 4 * NW        # 2048-col chunks: 0.5MB in-DMA, deep pipeline
            # start the first big h transfer before the tiny const loads
            ht0 = hp.tile([128, CHW], bf, tag="ht")
            nc.sync.dma_start(out=ht0[:], in_=d_hT[:, 0:CHW])
            w_sb = cp.tile([IN_DIM, OUT_DIM], bf)
            nc.sync.dma_start(out=w_sb[:], in_=d_W[:])
            wt_sb = cp.tile([OUT_DIM, IN_DIM], bf)
            nc.sync.dma_start(out=wt_sb[:], in_=d_WT[:])
            a_sb = cp.tile([OUT_DIM, 2], bf)
            nc.sync.dma_start(out=a_sb[:], in_=d_a2[:])

            waug = cp.tile([IN_DIM, OUT_DIM + 2], bf)
            nc.vector.tensor_copy(out=waug[:, 0:OUT_DIM], in_=w_sb[:])
            ws_ps = psw.tile([IN_DIM, 2], mybir.dt.float32, space="PSUM")
            nc.tensor.matmul(out=ws_ps[:], lhsT=wt_sb[:], rhs=a_sb[:],
                             start=True, stop=True)
            nc.vector.tensor_copy(out=waug[:, OUT_DIM:OUT_DIM + 2], in_=ws_ps[:])

            ci = 0
            for g0 in range(0, NPP, CHW):
                g1 = min(g0 + CHW, NPP)
                gw = g1 - g0
                if g0 == 0:
                    ht = ht0
                else:
                    ht = hp.tile([128, CHW], bf, tag="ht")
                    nc.sync.dma_start(out=ht[:, :gw], in_=d_hT[:, g0:g1])
                wh_sb = wo.tile([OUT_DIM + 2, CHW], bf, tag="wh")
                for c0 in range(0, gw, NW):
                    w = min(c0 + NW, gw) - c0
                    wh_ps = psp.tile([OUT_DIM + 2, NW], mybir.dt.float32,
                                     space="PSUM")
                    nc.tensor.matmul(out=wh_ps[:, :w], lhsT=waug[:],
                                     rhs=ht[:, c0:c0 + w], start=True, stop=True)
                    # alternate PSUM->SBUF copies across DVE and ACT
                    if ci % 2 == 0:
                        nc.vector.tensor_copy(out=wh_sb[:, c0:c0 + w],
                                              in_=wh_ps[:, :w])
                    else:
                        nc.scalar.activation(out=wh_sb[:, c0:c0 + w],
                                             in_=wh_ps[:, :w],
                                             func=mybir.ActivationFunctionType.Copy)
                    ci += 1
                nc.sync.dma_start(out=d_whT[:, g0:g1], in_=wh_sb[:, :gw])
    nc.compile()
    return nc


def _build_pass2(Tg, eps_free=False):
    from concourse import bacc, mybir
    import concourse.tile as tile

    bf = mybir.dt.bfloat16
    f32 = mybir.dt.float32
    i32 = mybir.dt.int32
    alu = mybir.AluOpType
    act = mybir.ActivationFunctionType

    base = np.zeros(NGRP + 1, np.int64)
    base[1:] = np.cumsum(Tg)
    TT = int(base[-1])          # total group-tiles per core
    TT4 = TT * NLANE            # total (tile, lane) columns

    nc = bacc.Bacc("TRN2", target_bir_lowering=False, debug=False)
    d_msg = nc.dram_tensor("msg", [128, TT4 * W65], bf, kind="ExternalInput")
    d_qr = nc.dram_tensor("qr", [128, 4 * TT4], bf, kind="ExternalInput")
    d_out = nc.dram_tensor("out", [128, NGRP * OUT_DIM], bf,
                           kind="ExternalOutput")

    n_sb = (NGRP + SBG - 1) // SBG
    maxT4 = max(int(base[min(s * SBG + SBG, NGRP)] - base[s * SBG])
                for s in range(n_sb)) * NLANE

    with tile.TileContext(nc) as tc:
        with tc.tile_pool(name="c2", bufs=1) as cp, \
             tc.tile_pool(name="gp", bufs=3) as gp, \
             tc.tile_pool(name="mp", bufs=3) as mp, \
             tc.tile_pool(name="fp", bufs=8) as fp, \
             tc.tile_pool(name="pp", bufs=8, space="PSUM") as pp:

            # tiny iota in bf16 (values < 256 exact); broadcast via AP view
            iota_f = cp.tile([128, BW], bf)
            nc.gpsimd.iota(iota_f[:], pattern=[[1, BW]], base=0,
                           channel_multiplier=0,
                           allow_small_or_imprecise_dtypes=True)

            # q2 | rr2 (pair-duplicated on host): load sb0's slices first
            qr_sb = cp.tile([128, 4 * TT4], bf)
            cut = int(base[SBG]) * NLANE * 2
            for lo, hi in ((0, cut), (cut, 2 * TT4)):
                for k in range(2):
                    nc.sync.dma_start(
                        out=qr_sb[:, k * 2 * TT4 + lo:k * 2 * TT4 + hi],
                        in_=d_qr[:, k * 2 * TT4 + lo:k * 2 * TT4 + hi])
            q2_sb = qr_sb[:, 0:2 * TT4]
            rr2_sb = qr_sb[:, 2 * TT4:4 * TT4]
            out_all = cp.tile([128, NGRP * OUT_DIM], bf)

            # ex2 = exp(leakyrelu(q2)), first superblock's slots first
            # (ACT Lrelu ignores alpha on HW; fused DVE (q*slope) max q)
            ex2_sb = cp.tile([128, 2 * TT4], bf)
            for a, b in ((0, cut), (cut, 2 * TT4)):
                nc.vector.scalar_tensor_tensor(
                    out=ex2_sb[:, a:b], in0=q2_sb[:, a:b], scalar=NEG_SLOPE,
                    in1=q2_sb[:, a:b], op0=alu.mult, op1=alu.max)
                nc.scalar.activation(out=ex2_sb[:, a:b], in_=ex2_sb[:, a:b],
                                     func=act.Exp)

            for s in range(n_sb):
                g0, g1 = s * SBG, min((s + 1) * SBG, NGRP)
                t0, t1 = int(base[g0]), int(base[g1])
                Ts = t1 - t0        # group-tiles in this superblock
                C = Ts * NLANE      # (tile, lane) columns
                G = gp.tile([128, C * W65], bf, tag="G")
                nc.sync.dma_start(out=G[:], in_=d_msg[:, t0 * NLANE * W65:
                                                      t1 * NLANE * W65])

                # M[p, (c, j)] = ex[p, c] * (iota_j == rr[p, c])
                # pair-duplicated rr2/ex2 + dense iota keep DVE in 2x_1P
                M = mp.tile([128, C * BW], bf, tag="M")
                m_v = M[:].rearrange("p (c a b) -> p c a b", a=BW // 2, b=2)
                io_v = iota_f[:].rearrange(
                    "p (o a b) -> p o a b", o=1, b=2).to_broadcast(
                    [128, C, BW // 2, 2])
                rr_v = rr2_sb[:, 2 * t0 * NLANE:2 * t1 * NLANE].rearrange(
                    "p (c o b) -> p c o b", o=1, b=2).to_broadcast(
                    [128, C, BW // 2, 2])
                ex_v = ex2_sb[:, 2 * t0 * NLANE:2 * t1 * NLANE].rearrange(
                    "p (c o b) -> p c o b", o=1, b=2).to_broadcast(
                    [128, C, BW // 2, 2])
                nc.vector.tensor_tensor(out=m_v, in0=io_v, in1=rr_v,
                                        op=alu.is_equal)
                nc.vector.tensor_tensor(out=m_v, in0=m_v, in1=ex_v,
                                        op=alu.mult)

                for g in range(g0, g1):
                    tg = int(Tg[g])
                    lb = int(base[g]) - t0
                    ps = pp.tile([128, W65], f32, space="PSUM", tag="ps")
                    for t in range(tg):
                        for k in range(NLANE):
                            nc.tensor.matmul(
                                out=ps[32 * k:32 * k + 32, :],
                                lhsT=M[:, (lb + t) * NLANE * BW + k * BW:
                                       (lb + t) * NLANE * BW + (k + 1) * BW],
                                rhs=G[:, ((lb + t) * NLANE + k) * W65:
                                      ((lb + t) * NLANE + k + 1) * W65],
                                start=(t == 0), stop=(t == tg - 1),
                                tile_position=(0, 32 * k))
                    dinv = fp.tile([128, 1], f32, tag="di")
                    if eps_free:
                        nc.vector.reciprocal(out=dinv[:], in_=ps[:, OUT_DIM:W65])
                    else:
                        dtmp = fp.tile([128, 1], f32, tag="dt")
                        nc.vector.tensor_scalar(out=dtmp[:],
                                                in0=ps[:, OUT_DIM:W65],
                                                scalar1=1e-10, scalar2=None,
                                                op0=alu.add)
                        nc.vector.reciprocal(out=dinv[:], in_=dtmp[:])
                    nc.scalar.activation(
                        out=out_all[:, g * OUT_DIM:(g + 1) * OUT_DIM],
                        in_=ps[:, 0:OUT_DIM], func=act.Copy, scale=dinv[:])
                # flush staged output every 6 superblocks (3 big DMAs)
                if s % 6 == 5 or s == n_sb - 1:
                    f0 = (s - s % 6) * SBG
                    nc.sync.dma_start(
                        out=d_out[:, f0 * OUT_DIM:g1 * OUT_DIM],
                        in_=out_all[:, f0 * OUT_DIM:g1 * OUT_DIM])
    nc.compile()
    return nc


def _prep_structure(row):
    """Bucket edges by 32-node dest block; deal sorted blocks onto
    (core, group, lane) so the 32 blocks sharing a group have similar
    counts; assign each edge a (core, partition, column) slot."""
    gb = row // BW                          # global block per edge (< 3125)
    cnt = np.bincount(gb, minlength=NGB)
    sorted_ids = np.argsort(-cnt, kind="stable")
    k = np.arange(NGB)
    blk_core = np.empty(NGB, np.int64)
    blk_grp = np.empty(NGB, np.int64)
    blk_lane = np.empty(NGB, np.int64)
    blk_core[sorted_ids] = k % CORES
    blk_grp[sorted_ids] = k // NLANES_G
    blk_lane[sorted_ids] = (k % NLANES_G) // CORES
    # per group: tiles = ceil(max count over its 32 blocks / 128)
    Tg = np.maximum(1, (cnt[sorted_ids[::NLANES_G]] + 127) // 128)
    base = np.zeros(NGRP + 1, np.int64)
    base[1:] = np.cumsum(Tg)
    TT = int(base[-1])

    key = (blk_core[gb] * NGRP + blk_grp[gb]) * NLANE + blk_lane[gb]
    kcnt = np.bincount(key, minlength=NGB)
    order = np.argsort(key, kind="stable")
    starts = np.zeros(NGB, np.int64)
    starts[1:] = np.cumsum(kcnt)[:-1]
    rank = np.arange(N_EDGES, dtype=np.int64) - np.repeat(starts, kcnt)
    key_s = key[order]
    core_s = key_s // (NGRP * NLANE)
    grp_s = (key_s // NLANE) % NGRP
    lane_s = key_s % NLANE
    t_loc = rank >> 7
    p_s = rank & 127
    col = (base[grp_s] + t_loc) * NLANE + lane_s   # (tile, lane) column
    return dict(order=order, core_s=core_s, p_s=p_s, col=col,
                gb_s=gb[order], Tg=Tg, base=base, TT=TT,
                blk_core=blk_core, blk_grp=blk_grp, blk_lane=blk_lane)


def _run_spmd(nc, in_maps, trace=False):
    from concourse import bass_utils
    res = bass_utils.run_bass_kernel_spmd(
        nc, in_maps, core_ids=list(range(CORES)), trace=trace)
    return res


def kernel(h, row, col, W, a):
    trace = bool(os.environ.get("GAT_TRACE"))
    if trace:
        try:
            import ntff_shim
            ntff_shim.install()
        except Exception:
            trace = False

    h = np.ascontiguousarray(np.asarray(h, dtype=np.float32))
    W = np.ascontiguousarray(np.asarray(W, dtype=np.float32))
    a = np.ascontiguousarray(np.asarray(a, dtype=np.float32)).reshape(2 * OUT_DIM)
    row = np.asarray(row).astype(np.int64)
    col = np.asarray(col).astype(np.int64)

    # ---- pass 1: Wh / s_src / s_dst, node-sharded, bf16 ----
    nc1 = _build_pass1()
    W16 = W.astype(BF16)
    WT16 = np.ascontiguousarray(W16.T)
    a2 = np.ascontiguousarray(
        np.stack([a[:OUT_DIM], a[OUT_DIM:]], axis=1)).astype(BF16)
    in_maps1 = []
    for c in range(CORES):
        hpad = np.zeros((NPP, IN_DIM), np.float32)
        hpad[:NPC] = h[c * NPC:(c + 1) * NPC]
        in_maps1.append({"hT": np.ascontiguousarray(hpad.T).astype(BF16),
                         "Wm": W16, "WT": WT16, "a2": a2})
    res1 = _run_spmd(nc1, in_maps1, trace=trace)
    if trace:
        LAST_STATS["pass1_ns"] = res1.exec_time_ns

    WhA = np.ones((N_NODES, W65), BF16)
    s_src = np.empty(N_NODES, np.float32)
    s_dst = np.empty(N_NODES, np.float32)
    for c in range(CORES):
        whT = res1.results[c]["whT"]
        WhA[c * NPC:(c + 1) * NPC, :OUT_DIM] = whT[:OUT_DIM, :NPC].T
        s_src[c * NPC:(c + 1) * NPC] = whT[OUT_DIM, :NPC].astype(np.float32)
        s_dst[c * NPC:(c + 1) * NPC] = whT[OUT_DIM + 1, :NPC].astype(np.float32)

    # ---- host: edge-slot structure + gathered bf16 streams ----
    st = _prep_structure(row)
    Tg, TT = st["Tg"], st["TT"]
    TT4 = TT * NLANE
    cs, ps, cc = st["core_s"], st["p_s"], st["col"]
    row_s = row[st["order"]]
    col_s = col[st["order"]]

    msg = np.zeros((CORES, 128, TT4, W65), BF16)
    msg[cs, ps, cc] = WhA[col_s]
    q = np.full((CORES, 128, TT4), PAD_Q, np.float32)
    q[cs, ps, cc] = s_src[row_s] + s_dst[col_s]
    rr = np.zeros((CORES, 128, TT4), np.float32)
    rr[cs, ps, cc] = (row_s - st["gb_s"] * BW).astype(np.float32)
    q2 = np.repeat(q, 2, axis=2).astype(BF16)
    rr2 = np.repeat(rr, 2, axis=2).astype(BF16)

    # ---- pass 2: attention + segment sum ----
    eps_free = int(np.bincount(row, minlength=N_NODES).min()) > 0
    nc2 = _build_pass2(Tg, eps_free=eps_free)
    in_maps2 = [{"msg": msg[c].reshape(128, TT4 * W65),
                 "qr": np.concatenate([q2[c], rr2[c]], axis=1)}
                for c in range(CORES)]
    res2 = _run_spmd(nc2, in_maps2, trace=trace)
    if trace:
        LAST_STATS["pass2_ns"] = res2.exec_time_ns
        LAST_STATS["total_ns"] = (res1.exec_time_ns or 0) + (res2.exec_time_ns or 0)

    # ---- host: un-permute node-partitioned outputs ----
    out = np.empty((N_NODES, OUT_DIM), np.float32)
    NGB_REAL = N_NODES // BW    # 3125, exact
    inv_core = st["blk_core"][:NGB_REAL]
    inv_grp = st["blk_grp"][:NGB_REAL]
    inv_lane = st["blk_lane"][:NGB_REAL]
    devs = [np.asarray(res2.results[c]["out"]).astype(np.float32)
            .reshape(NLANE, BW, NGRP, OUT_DIM) for c in range(CORES)]
    blocks = np.arange(NGB_REAL)
    for c in range(CORES):
        sel = inv_core == c
        b = blocks[sel]
        out.reshape(NGB_REAL, BW, OUT_DIM)[b] = \
            devs[c][inv_lane[sel], :, inv_grp[sel]].transpose(0, 1, 2)
    return out


# revision 20
# speedup vs baseline: 1.0053x; 1.0053x over previous
"""GAT influence layer on 8 Trainium2 NeuronCores (Bass/Tile), bf16.

Strategy (edge-parallel, dest-block sharded):
  Pass 1 (device): each core computes its 12.5k-node slice of
      Wh = h @ W, s_src = Wh @ a_src, s_dst = Wh @ a_dst  (bf16 matmuls
      against an augmented weight matrix; bf16 in/out streams).
  Host: buckets edges by 32-node destination block; permutes blocks onto
      (core, group, lane) so the 32 blocks sharing a group index have
      similar edge counts; builds per-core bf16 streams: gathered
      messages WhA[col] (65 wide incl. a ones column for the softmax
      denominator), pair-duplicated attention logits q=s_src[row]+
      s_dst[col], and pair-duplicated within-block row indices.
      Data movement only.
  Pass 2 (device): exp(leakyrelu(q)) on ScalarE; a batched one-hot
      selection matrix M = (iota==rr)*exp via two DVE tensor_tensor ops
      (pair-duplicated operands keep the 2x_1P perf mode); the
      softmax-weighted segment-sum as PSUM-accumulated TensorE matmuls,
      4 blocks per 128-row PSUM tile via col-tiling (tile_position);
      deferred division by the per-node denominator (the reference's
      global max-subtract cancels analytically in the softmax).
  Host: un-permutes per-core node-partitioned outputs.
"""

import os
import numpy as np
import ml_dtypes

BF16 = ml_dtypes.bfloat16

N_NODES = 100000
N_EDGES = 1600000
IN_DIM = 128
OUT_DIM = 64
NEG_SLOPE = 0.2
CORES = 8
NPC = N_NODES // CORES          # nodes per core (12500)
BW = 32                         # nodes per block
NLANE = 4                       # blocks (lanes) per group = one PSUM tile
NGRP = 98                       # groups per core; 8*98*4*32 = 100352 >= 1e5
NLANES_G = CORES * NLANE        # 32 blocks share one group index
NGB = CORES * NGRP * NLANE      # 3136 global lane slots (3125 real)
NPP = 12544                     # padded nodes per core, pass 1 (98*128)
SBG = 8                         # groups per superblock (pass-2 stage)
FLUSH = 4                       # superblocks per output-flush window
PAD_Q = -30000.0                # pad-slot logit -> exp == 0
W65 = OUT_DIM + 1

LAST_STATS = {}


def _build_pass1():
    from concourse import bacc, mybir
    import concourse.tile as tile

    bf = mybir.dt.bfloat16
    nc = bacc.Bacc("TRN2", target_bir_lowering=False, debug=False)
    d_hT = nc.dram_tensor("hT", [128, NPP], bf, kind="ExternalInput")
    d_W = nc.dram_tensor("Wm", [IN_DIM, OUT_DIM], bf, kind="ExternalInput")
    d_WT = nc.dram_tensor("WT", [OUT_DIM, IN_DIM], bf, kind="ExternalInput")
    d_a2 = nc.dram_tensor("a2", [OUT_DIM, 2], bf, kind="ExternalInput")
    d_whT = nc.dram_tensor("whT", [OUT_DIM + 2, NPP], bf, kind="ExternalOutput")

    NW = 512                    # moving-operand chunk (one PSUM bank fp32)
    with tile.TileContext(nc) as tc:
        with tc.tile_pool(name="c1", bufs=1) as cp, \
             tc.tile_pool(name="ht1", bufs=4) as hp, \
             tc.tile_pool(name="wo1", bufs=4) as wo, \
             tc.tile_pool(name="psw", bufs=1, space="PSUM") as psw, \
             tc.tile_pool(name="ps1", bufs=6, space="PSUM") as psp:
            CHW = 4 * NW        # 2048-col chunks: 0.5MB in-DMA, deep pipeline
            # start the first big h transfer before the tiny const loads
            ht0 = hp.tile([128, CHW], bf, tag="ht")
            nc.sync.dma_start(out=ht0[:], in_=d_hT[:, 0:CHW])
            w_sb = cp.tile([IN_DIM, OUT_DIM], bf)
            nc.sync.dma_start(out=w_sb[:], in_=d_W[:])
            wt_sb = cp.tile([OUT_DIM, IN_DIM], bf)
            nc.sync.dma_start(out=wt_sb[:], in_=d_WT[:])
            a_sb = cp.tile([OUT_DIM, 2], bf)
            nc.sync.dma_start(out=a_sb[:], in_=d_a2[:])

            waug = cp.tile([IN_DIM, OUT_DIM + 2], bf)
            nc.vector.tensor_copy(out=waug[:, 0:OUT_DIM], in_=w_sb[:])
            ws_ps = psw.tile([IN_DIM, 2], mybir.dt.float32, space="PSUM")
            nc.tensor.matmul(out=ws_ps[:], lhsT=wt_sb[:], rhs=a_sb[:],
                             start=True, stop=True)
            nc.vector.tensor_copy(out=waug[:, OUT_DIM:OUT_DIM + 2], in_=ws_ps[:])

            ci = 0
            for g0 in range(0, NPP, CHW):
                g1 = min(g0 + CHW, NPP)
                gw = g1 - g0
                if g0 == 0:
                    ht = ht0
                else:
                    ht = hp.tile([128, CHW], bf, tag="ht")
                    nc.sync.dma_start(out=ht[:, :gw], in_=d_hT[:, g0:g1])
                wh_sb = wo.tile([OUT_DIM + 2, CHW], bf, tag="wh")
                for c0 in range(0, gw, NW):
                    w = min(c0 + NW, gw) - c0
                    wh_ps = psp.tile([OUT_DIM + 2, NW], mybir.dt.float32,
                                     space="PSUM")
                    nc.tensor.matmul(out=wh_ps[:, :w], lhsT=waug[:],
                                     rhs=ht[:, c0:c0 + w], start=True, stop=True)
                    # alternate PSUM->SBUF copies across DVE and ACT
                    if ci % 2 == 0:
                        nc.vector.tensor_copy(out=wh_sb[:, c0:c0 + w],
                                              in_=wh_ps[:, :w])
                    else:
                        nc.scalar.activation(out=wh_sb[:, c0:c0 + w],
                                             in_=wh_ps[:, :w],
                                             func=mybir.ActivationFunctionType.Copy)
                    ci += 1
                nc.sync.dma_start(out=d_whT[:, g0:g1], in_=wh_sb[:, :gw])
    nc.compile()
    return nc


def _build_pass2(Tg, eps_free=False):
    from concourse import bacc, mybir
    import concourse.tile as tile

    bf = mybir.dt.bfloat16
    f32 = mybir.dt.float32
    i32 = mybir.dt.int32
    alu = mybir.AluOpType
    act = mybir.ActivationFunctionType

    base = np.zeros(NGRP + 1, np.int64)
    base[1:] = np.cumsum(Tg)
    TT = int(base[-1])          # total group-tiles per core
    TT4 = TT * NLANE            # total (tile, lane) columns

    nc = bacc.Bacc("TRN2", target_bir_lowering=False, debug=False)
    d_msg = nc.dram_tensor("msg", [128, TT4 * W65], bf, kind="ExternalInput")
    d_qr = nc.dram_tensor("qr", [128, 4 * TT4], bf, kind="ExternalInput")
    d_out = nc.dram_tensor("out", [128, NGRP * OUT_DIM], bf,
                           kind="ExternalOutput")

    n_sb = (NGRP + SBG - 1) // SBG
    maxT4 = max(int(base[min(s * SBG + SBG, NGRP)] - base[s * SBG])
                for s in range(n_sb)) * NLANE

    with tile.TileContext(nc) as tc:
        with tc.tile_pool(name="c2", bufs=1) as cp, \
             tc.tile_pool(name="gp", bufs=3) as gp, \
             tc.tile_pool(name="mp", bufs=3) as mp, \
             tc.tile_pool(name="fp", bufs=8) as fp, \
             tc.tile_pool(name="op", bufs=2) as op, \
             tc.tile_pool(name="pp", bufs=8, space="PSUM") as pp:

            # tiny iota in bf16 (values < 256 exact); broadcast via AP view
            iota_f = cp.tile([128, BW], bf)
            nc.gpsimd.iota(iota_f[:], pattern=[[1, BW]], base=0,
                           channel_multiplier=0,
                           allow_small_or_imprecise_dtypes=True)

            # q2 | rr2 (pair-duplicated on host): load sb0's slices first
            qr_sb = cp.tile([128, 4 * TT4], bf)
            cut = int(base[SBG]) * NLANE * 2
            for lo, hi in ((0, cut), (cut, 2 * TT4)):
                for k in range(2):
                    nc.sync.dma_start(
                        out=qr_sb[:, k * 2 * TT4 + lo:k * 2 * TT4 + hi],
                        in_=d_qr[:, k * 2 * TT4 + lo:k * 2 * TT4 + hi])
            q2_sb = qr_sb[:, 0:2 * TT4]
            rr2_sb = qr_sb[:, 2 * TT4:4 * TT4]

            # ex2 = exp(leakyrelu(q2)), first superblock's slots first
            # (ACT Lrelu ignores alpha on HW; fused DVE (q*slope) max q)
            ex2_sb = cp.tile([128, 2 * TT4], bf)
            for a, b in ((0, cut), (cut, 2 * TT4)):
                nc.vector.scalar_tensor_tensor(
                    out=ex2_sb[:, a:b], in0=q2_sb[:, a:b], scalar=NEG_SLOPE,
                    in1=q2_sb[:, a:b], op0=alu.mult, op1=alu.max)
                nc.scalar.activation(out=ex2_sb[:, a:b], in_=ex2_sb[:, a:b],
                                     func=act.Exp)

            out_win = None
            win_g0 = 0
            for s in range(n_sb):
                g0, g1 = s * SBG, min((s + 1) * SBG, NGRP)
                if s % FLUSH == 0:
                    out_win = op.tile([128, FLUSH * SBG * OUT_DIM], bf,
                                      tag="ow")
                    win_g0 = g0
                t0, t1 = int(base[g0]), int(base[g1])
                Ts = t1 - t0        # group-tiles in this superblock
                C = Ts * NLANE      # (tile, lane) columns
                G = gp.tile([128, C * W65], bf, tag="G")
                nc.sync.dma_start(out=G[:], in_=d_msg[:, t0 * NLANE * W65:
                                                      t1 * NLANE * W65])

                # M[p, (c, j)] = ex[p, c] * (iota_j == rr[p, c])
                # pair-duplicated rr2/ex2 + dense iota keep DVE in 2x_1P
                M = mp.tile([128, C * BW], bf, tag="M")
                m_v = M[:].rearrange("p (c a b) -> p c a b", a=BW // 2, b=2)
                io_v = iota_f[:].rearrange(
                    "p (o a b) -> p o a b", o=1, b=2).to_broadcast(
                    [128, C, BW // 2, 2])
                rr_v = rr2_sb[:, 2 * t0 * NLANE:2 * t1 * NLANE].rearrange(
                    "p (c o b) -> p c o b", o=1, b=2).to_broadcast(
                    [128, C, BW // 2, 2])
                ex_v = ex2_sb[:, 2 * t0 * NLANE:2 * t1 * NLANE].rearrange(
                    "p (c o b) -> p c o b", o=1, b=2).to_broadcast(
                    [128, C, BW // 2, 2])
                nc.vector.tensor_tensor(out=m_v, in0=io_v, in1=rr_v,
                                        op=alu.is_equal)
                nc.vector.tensor_tensor(out=m_v, in0=m_v, in1=ex_v,
                                        op=alu.mult)

                for g in range(g0, g1):
                    tg = int(Tg[g])
                    lb = int(base[g]) - t0
                    ps = pp.tile([128, W65], f32, space="PSUM", tag="ps")
                    for t in range(tg):
                        for k in range(NLANE):
                            nc.tensor.matmul(
                                out=ps[32 * k:32 * k + 32, :],
                                lhsT=M[:, (lb + t) * NLANE * BW + k * BW:
                                       (lb + t) * NLANE * BW + (k + 1) * BW],
                                rhs=G[:, ((lb + t) * NLANE + k) * W65:
                                      ((lb + t) * NLANE + k + 1) * W65],
                                start=(t == 0), stop=(t == tg - 1),
                                tile_position=(0, 32 * k))
                    dinv = fp.tile([128, 1], f32, tag="di")
                    if eps_free:
                        nc.vector.reciprocal(out=dinv[:], in_=ps[:, OUT_DIM:W65])
                    else:
                        dtmp = fp.tile([128, 1], f32, tag="dt")
                        nc.vector.tensor_scalar(out=dtmp[:],
                                                in0=ps[:, OUT_DIM:W65],
                                                scalar1=1e-10, scalar2=None,
                                                op0=alu.add)
                        nc.vector.reciprocal(out=dinv[:], in_=dtmp[:])
                    gr = g - win_g0
                    nc.scalar.activation(
                        out=out_win[:, gr * OUT_DIM:(gr + 1) * OUT_DIM],
                        in_=ps[:, 0:OUT_DIM], func=act.Copy, scale=dinv[:])
                # flush the staged output window (rotating tiles: no WAR)
                if s % FLUSH == FLUSH - 1 or s == n_sb - 1:
                    nc.sync.dma_start(
                        out=d_out[:, win_g0 * OUT_DIM:g1 * OUT_DIM],
                        in_=out_win[:, 0:(g1 - win_g0) * OUT_DIM])
    nc.compile()
    return nc


def _prep_structure(row):
    """Bucket edges by 32-node dest block; deal sorted blocks onto
    (core, group, lane) so the 32 blocks sharing a group have similar
    counts; assign each edge a (core, partition, column) slot."""
    gb = row // BW                          # global block per edge (< 3125)
    cnt = np.bincount(gb, minlength=NGB)
    sorted_ids = np.argsort(-cnt, kind="stable")
    k = np.arange(NGB)
    blk_core = np.empty(NGB, np.int64)
    blk_grp = np.empty(NGB, np.int64)
    blk_lane = np.empty(NGB, np.int64)
    blk_core[sorted_ids] = k % CORES
    blk_grp[sorted_ids] = k // NLANES_G
    blk_lane[sorted_ids] = (k % NLANES_G) // CORES
    # per group: tiles = ceil(max count over its 32 blocks / 128)
    Tg = np.maximum(1, (cnt[sorted_ids[::NLANES_G]] + 127) // 128)
    base = np.zeros(NGRP + 1, np.int64)
    base[1:] = np.cumsum(Tg)
    TT = int(base[-1])

    key = (blk_core[gb] * NGRP + blk_grp[gb]) * NLANE + blk_lane[gb]
    kcnt = np.bincount(key, minlength=NGB)
    order = np.argsort(key, kind="stable")
    starts = np.zeros(NGB, np.int64)
    starts[1:] = np.cumsum(kcnt)[:-1]
    rank = np.arange(N_EDGES, dtype=np.int64) - np.repeat(starts, kcnt)
    key_s = key[order]
    core_s = key_s // (NGRP * NLANE)
    grp_s = (key_s // NLANE) % NGRP
    lane_s = key_s % NLANE
    t_loc = rank >> 7
    p_s = rank & 127
    col = (base[grp_s] + t_loc) * NLANE + lane_s   # (tile, lane) column
    return dict(order=order, core_s=core_s, p_s=p_s, col=col,
                gb_s=gb[order], Tg=Tg, base=base, TT=TT,
                blk_core=blk_core, blk_grp=blk_grp, blk_lane=blk_lane)


def _run_spmd(nc, in_maps, trace=False):
    from concourse import bass_utils
    res = bass_utils.run_bass_kernel_spmd(
        nc, in_maps, core_ids=list(range(CORES)), trace=trace)
    return res


def kernel(h, row, col, W, a):
    trace = bool(os.environ.get("GAT_TRACE"))
    if trace:
        try:
            import ntff_shim
            ntff_shim.install()
        except Exception:
            trace = False

    h = np.ascontiguousarray(np.asarray(h, dtype=np.float32))
    W = np.ascontiguousarray(np.asarray(W, dtype=np.float32))
    a = np.ascontiguousarray(np.asarray(a, dtype=np.float32)).reshape(2 * OUT_DIM)
    row = np.asarray(row).astype(np.int64)
    col = np.asarray(col).astype(np.int64)

    # ---- pass 1: Wh / s_src / s_dst, node-sharded, bf16 ----
    nc1 = _build_pass1()
    W16 = W.astype(BF16)
    WT16 = np.ascontiguousarray(W16.T)
    a2 = np.ascontiguousarray(
        np.stack([a[:OUT_DIM], a[OUT_DIM:]], axis=1)).astype(BF16)
    in_maps1 = []
    for c in range(CORES):
        hpad = np.zeros((NPP, IN_DIM), np.float32)
        hpad[:NPC] = h[c * NPC:(c + 1) * NPC]
        in_maps1.append({"hT": np.ascontiguousarray(hpad.T).astype(BF16),
                         "Wm": W16, "WT": WT16, "a2": a2})
    res1 = _run_spmd(nc1, in_maps1, trace=trace)
    if trace:
        LAST_STATS["pass1_ns"] = res1.exec_time_ns

    WhA = np.ones((N_NODES, W65), BF16)
    s_src = np.empty(N_NODES, np.float32)
    s_dst = np.empty(N_NODES, np.float32)
    for c in range(CORES):
        whT = res1.results[c]["whT"]
        WhA[c * NPC:(c + 1) * NPC, :OUT_DIM] = whT[:OUT_DIM, :NPC].T
        s_src[c * NPC:(c + 1) * NPC] = whT[OUT_DIM, :NPC].astype(np.float32)
        s_dst[c * NPC:(c + 1) * NPC] = whT[OUT_DIM + 1, :NPC].astype(np.float32)

    # ---- host: edge-slot structure + gathered bf16 streams ----
    st = _prep_structure(row)
    Tg, TT = st["Tg"], st["TT"]
    TT4 = TT * NLANE
    cs, ps, cc = st["core_s"], st["p_s"], st["col"]
    row_s = row[st["order"]]
    col_s = col[st["order"]]

    msg = np.zeros((CORES, 128, TT4, W65), BF16)
    msg[cs, ps, cc] = WhA[col_s]
    q = np.full((CORES, 128, TT4), PAD_Q, np.float32)
    q[cs, ps, cc] = s_src[row_s] + s_dst[col_s]
    rr = np.zeros((CORES, 128, TT4), np.float32)
    rr[cs, ps, cc] = (row_s - st["gb_s"] * BW).astype(np.float32)
    q2 = np.repeat(q, 2, axis=2).astype(BF16)
    rr2 = np.repeat(rr, 2, axis=2).astype(BF16)

    # ---- pass 2: attention + segment sum ----
    eps_free = int(np.bincount(row, minlength=N_NODES).min()) > 0
    nc2 = _build_pass2(Tg, eps_free=eps_free)
    in_maps2 = [{"msg": msg[c].reshape(128, TT4 * W65),
                 "qr": np.concatenate([q2[c], rr2[c]], axis=1)}
                for c in range(CORES)]
    res2 = _run_spmd(nc2, in_maps2, trace=trace)
    if trace:
        LAST_STATS["pass2_ns"] = res2.exec_time_ns
        LAST_STATS["total_ns"] = (res1.exec_time_ns or 0) + (res2.exec_time_ns or 0)

    # ---- host: un-permute node-partitioned outputs ----
    out = np.empty((N_NODES, OUT_DIM), np.float32)
    NGB_REAL = N_NODES // BW    # 3125, exact
    inv_core = st["blk_core"][:NGB_REAL]
    inv_grp = st["blk_grp"][:NGB_REAL]
    inv_lane = st["blk_lane"][:NGB_REAL]
    devs = [np.asarray(res2.results[c]["out"]).astype(np.float32)
            .reshape(NLANE, BW, NGRP, OUT_DIM) for c in range(CORES)]
    blocks = np.arange(NGB_REAL)
    for c in range(CORES):
        sel = inv_core == c
        b = blocks[sel]
        out.reshape(NGB_REAL, BW, OUT_DIM)[b] = \
            devs[c][inv_lane[sel], :, inv_grp[sel]].transpose(0, 1, 2)
    return out
